# revision 10
# baseline (speedup 1.0000x reference)
"""Trainium2 Bass kernel for nn_BasicEncoder (embedding-lookup encoder).

reference math:
    counts[b, v] = histogram of the 512 token ids in row b          [B, V]
    h      = relu(counts @ enc1_w.T + enc1_b)                       [B, 16]
    mean   = h @ mean_w.T + mean_b                                  [B, 16]
    logvar = h @ logvar_w.T + logvar_b                              [B, 16]

Key identity: counts @ enc1_w.T == sum_s enc1_w[:, x[b, s]] — a gather-and-
sum of embedding-table columns; the [B, V] histogram is never materialized.

Device strategy (data-parallel over 8 NeuronCores, 256 rows x 512 tokens
per core):

  - The gather runs on the SWDGE DMA path (InstDMAGatherAnt): the Q7 cores
    generate one 64B descriptor per token (table row = 16 f32), and the 16
    DMA engines execute them HBM -> SBUF.  This replaces the Q7 ap_gather
    inner loop (the baseline), which is limited to ~2 indices per
    ~102-cycle SBUF read command.  The table lives in HBM as [V, 64] f32
    (256B row stride — the descriptor base-address step granularity); only
    bytes 0:64 of each row are fetched, so the pad is never read.
  - The runtime's SWDGE descriptor ring caps one gather instruction at
    ~65..120 descriptors per DMA engine; chunks of NI indices (NI/16+1
    descs/ring) stay under it.  Chunks rotate across the 4 SWDGE queues.
  - dma_gather's fixed dst layout: stream position i -> partition i%128,
    slot i//128.  Rows are mapped to partitions (128-row waves), so slot j
    of partition p holds token j's embedding for row p.  A strided DVE
    reduce over slots yields h-partials [128 rows, 16].
  - Tail per wave: PE transpose ([128,16] -> psum [16,128]), scalar Relu
    with per-partition bias enc1_b, then mean/logvar as one matmul each
    with a ones-row-augmented stationary (bias folded into the matmul).
  - Output [128 rows, 32] per wave, assembled host-side.
"""

import numpy as np

B, S, V, H, O = 2048, 512, 32128, 16, 16
NCORES = 8
RPC = B // NCORES  # rows per core (256)
P = 128
NWAVES = RPC // P  # 128-row waves per core (2)
JPC = 8  # token slots per chunk
CPW = S // JPC  # gather chunks per wave (64)
NCHUNK = NWAVES * CPW  # gather chunks per core (128)
NI = JPC * P  # indices per chunk (1024)
NIW = NI // 16  # wrapped idx columns per chunk (64)
TSTRIDE = 64  # table row stride in f32 elements (256B)
NQ = 4  # SWDGE queues to rotate across

_CACHE = {}


def _dma_gather_raw(gp, out_ap, in_ap, idxs_ap, num_idxs, elem_size, elem_step,
                    queue_num=0):
    """nc.gpsimd.dma_gather without the elem_size_bytes %256 assert.

    The 256B-multiple restriction in the bass helper is only required for
    transpose mode (the SBUF xbar sprays 256B); the non-transpose ucode path
    (dma_gather.cpp gen_descs) supports arbitrary packet sizes.  Only the
    descriptor base-address stride (elem_step) must be a 256B multiple.
    """
    import concourse.mybir as mybir

    assert idxs_ap.dtype == mybir.dt.int16
    assert in_ap.dtype == out_ap.dtype
    stride_bytes = elem_step * mybir.dt.size(in_ap.dtype)
    stride_bytes_256, rem = divmod(stride_bytes, 256)
    assert rem == 0 and 0 < stride_bytes_256 < 256
    _in_ap = gp.lower_ap_dma(in_ap, for_custom_bir_dma=True)
    inst = gp.add_instruction(
        mybir.InstDMAGatherAnt(
            name=gp.bass.get_next_instruction_name(),
            ins=[
                *_in_ap,
                gp.lower_ap(idxs_ap),
                gp.lower_val_access(gp.to_reg(num_idxs)),
            ],
            outs=[gp.lower_ap(out_ap)],
            transpose=False,
            num_idxs=num_idxs,
            elem_size=elem_size,
            stride_bytes_256=stride_bytes_256,
            gen_mode=0,
            single_packet=True,
            queue_num=queue_num,
            sbuf_tokens_per_rank=0,
            sbuf_free_dim_per_rank=0,
            sbuf_free_dim_pad_per_rank=0,
            sbuf_byte_offset=0,
        )
    )
    return inst


def _build_nc(repeat=1):
    import contextlib

    import concourse.bacc as bacc
    import concourse.bass as bass
    import concourse.mybir as mybir
    import concourse.tile as tile

    f32 = mybir.dt.float32
    i16 = mybir.dt.int16
    nc = bacc.Bacc(
        None,
        target_bir_lowering=False,
        dynamic_dma_scratch_size=65536,
        num_swdge_queues=NQ,
    )

    tbl_d = nc.dram_tensor("tbl64", [V, TSTRIDE], f32, kind="ExternalInput")
    xi_d = nc.dram_tensor("xi16", [P, NCHUNK * NIW], i16, kind="ExternalInput")
    b1_d = nc.dram_tensor("b1col", [H, 1], f32, kind="ExternalInput")
    ident_d = nc.dram_tensor("ident", [P, P], f32, kind="ExternalInput")
    ones_d = nc.dram_tensor("onesrow", [1, P], f32, kind="ExternalInput")
    wm_d = nc.dram_tensor("wmaug", [H + 1, O], f32, kind="ExternalInput")
    wl_d = nc.dram_tensor("wlaug", [H + 1, O], f32, kind="ExternalInput")
    out_d = nc.dram_tensor("out", [P, NWAVES * 2 * O], f32, kind="ExternalOutput")

    with tile.TileContext(nc) as tc:
        with (
            tc.tile_pool(name="sb", bufs=1) as pool,
            tc.tile_pool(name="gth", bufs=4) as gpool,
            tc.tile_pool(name="pst", bufs=2, space=bass.MemorySpace.PSUM) as ptpool,
            tc.tile_pool(name="ps", bufs=1, space=bass.MemorySpace.PSUM) as pspool,
            tc.For_i(0, repeat, 1) if repeat > 1 else contextlib.nullcontext(),
        ):
            # -- small constants ------------------------------------------
            b1_sb = pool.tile([H, 1], f32)
            nc.sync.dma_start(b1_sb[:], b1_d[:])
            ident_sb = pool.tile([P, P], f32)
            nc.sync.dma_start(ident_sb[:], ident_d[:])
            wm_sb = pool.tile([H + 1, O], f32)
            nc.sync.dma_start(wm_sb[:], wm_d[:])
            wl_sb = pool.tile([H + 1, O], f32)
            nc.sync.dma_start(wl_sb[:], wl_d[:])

            # index streams, 8 DMAs so early gathers start promptly
            NIDMA = 8
            xi_sb = pool.tile([P, NCHUNK * NIW], i16)
            cols = NCHUNK * NIW // NIDMA
            for k in range(NIDMA):
                nc.sync.dma_start(
                    xi_sb[:, k * cols : (k + 1) * cols],
                    xi_d[:, k * cols : (k + 1) * cols],
                )

            # ones rows for the bias-augmented stationaries (one per wave)
            hrT = [pool.tile([H + 1, P], f32, name=f"hrT{w}") for w in range(NWAVES)]
            for w in range(NWAVES):
                nc.sync.dma_start(hrT[w][H : H + 1, :], ones_d[:])

            # -- gather + reduce ------------------------------------------
            hpart = [
                pool.tile([P, CPW * H], f32, name=f"hpart{w}") for w in range(NWAVES)
            ]
            for k in range(NCHUNK):
                w, c = divmod(k, CPW)
                gath = gpool.tile([P, JPC * H], f32)
                _dma_gather_raw(
                    nc.gpsimd,
                    out_ap=gath[:],
                    in_ap=tbl_d[:, 0:H],
                    idxs_ap=xi_sb[:, k * NIW : (k + 1) * NIW],
                    num_idxs=NI,
                    elem_size=H,
                    elem_step=TSTRIDE,
                    queue_num=k % NQ,
                )
                nc.vector.tensor_reduce(
                    out=hpart[w][:, c * H : (c + 1) * H],
                    in_=gath[:].rearrange("p (j s) -> p s j", s=H),
                    axis=mybir.AxisListType.X,
                    op=mybir.AluOpType.add,
                )

            # -- per-wave tail --------------------------------------------
            out_sb = pool.tile([P, NWAVES * 2 * O], f32)
            om_ps = pspool.tile([P, NWAVES * 2 * O], f32)
            for w in range(NWAVES):
                hsum = pool.tile([P, H], f32)
                nc.vector.tensor_reduce(
                    out=hsum[:],
                    in_=hpart[w][:].rearrange("p (c s) -> p s c", s=H),
                    axis=mybir.AxisListType.X,
                    op=mybir.AluOpType.add,
                )
                ht_ps = ptpool.tile([H, P], f32)
                nc.tensor.transpose(ht_ps[:], hsum[:], ident_sb[:])
                nc.scalar.activation(
                    out=hrT[w][0:H, :],
                    in_=ht_ps[:],
                    func=mybir.ActivationFunctionType.Relu,
                    bias=b1_sb[:],
                )
                nc.tensor.matmul(
                    om_ps[:, (2 * w) * O : (2 * w + 1) * O], hrT[w][:], wm_sb[:]
                )
                nc.tensor.matmul(
                    om_ps[:, (2 * w + 1) * O : (2 * w + 2) * O], hrT[w][:], wl_sb[:]
                )

            nc.vector.tensor_scalar(
                out=out_sb[:],
                in0=om_ps[:],
                scalar1=0.0,
                scalar2=None,
                op0=mybir.AluOpType.add,
            )
            nc.sync.dma_start(out_d[:], out_sb[:])

    nc.compile()
    return nc


def _get_nc(repeat=1):
    key = ("nc", repeat)
    if key not in _CACHE:
        _CACHE[key] = _build_nc(repeat)
    return _CACHE[key]


def _prep_inputs(x, enc1_w, enc1_b, mean_w, mean_b, logvar_w, logvar_b):
    x = np.asarray(x)
    assert x.shape == (B, S)
    # stream position i of chunk (w, c): slot j = i // 128, partition
    # p = i % 128 -> token x[core_base + w*128 + p, c*JPC + j]
    xs = x.astype(np.int16).reshape(NCORES, NWAVES, P, CPW, JPC)  # [n, w, p, c, j]
    stream = xs.transpose(0, 1, 3, 4, 2)  # [n, w, c, j, p]
    stream = stream.reshape(NCORES, NCHUNK, NI)
    # idx i of a chunk is read from [16g + i%16, i//16] (any 16-partition
    # group g; the stream is replicated across all 8 groups)
    wrapped = stream.reshape(NCORES, NCHUNK, NIW, 16).transpose(0, 3, 1, 2)
    xi16 = np.tile(
        wrapped.reshape(NCORES, 1, 16, NCHUNK * NIW), (1, 8, 1, 1)
    ).reshape(NCORES, P, NCHUNK * NIW)
    xi16 = np.ascontiguousarray(xi16)

    tbl64 = np.zeros((V, TSTRIDE), dtype=np.float32)
    tbl64[:, :H] = np.asarray(enc1_w, dtype=np.float32).T  # [V, H]

    b1col = np.asarray(enc1_b, dtype=np.float32)[:, None].copy()
    ident = np.eye(P, dtype=np.float32)
    onesrow = np.ones((1, P), dtype=np.float32)

    def aug(wt, bias):  # [O, H] -> [H+1, O] with bias row
        a = np.empty((H + 1, O), dtype=np.float32)
        a[:H] = np.asarray(wt, dtype=np.float32).T
        a[H] = np.asarray(bias, dtype=np.float32)
        return a

    wmaug = aug(mean_w, mean_b)
    wlaug = aug(logvar_w, logvar_b)
    return [
        {
            "tbl64": tbl64,
            "xi16": xi16[c],
            "b1col": b1col,
            "ident": ident,
            "onesrow": onesrow,
            "wmaug": wmaug,
            "wlaug": wlaug,
        }
        for c in range(NCORES)
    ]


def _unscramble(out_core):
    # out_core [128, NWAVES*2*O]: partition p, wave w -> row w*128 + p
    o = out_core.reshape(P, NWAVES, 2, O)  # [p, w, {m,l}, o]
    o = o.transpose(2, 1, 0, 3)  # [{m,l}, w, p, o]
    return o.reshape(2, NWAVES * P, O)  # rows r = w*128 + p


def _run(in_maps, trace=False, repeat=1):
    from concourse.bass_utils import run_bass_kernel_spmd

    nc = _get_nc(repeat)
    core_ids = list(range(NCORES))
    res = run_bass_kernel_spmd(nc, in_maps, core_ids, trace=trace)
    mean = np.empty((B, O), dtype=np.float32)
    logvar = np.empty((B, O), dtype=np.float32)
    for c in core_ids:
        ml = _unscramble(res.results[c]["out"])
        mean[c * RPC : (c + 1) * RPC] = ml[0]
        logvar[c * RPC : (c + 1) * RPC] = ml[1]
    return mean, logvar, res


def kernel(x, enc1_w, enc1_b, mean_w, mean_b, logvar_w, logvar_b):
    in_maps = _prep_inputs(x, enc1_w, enc1_b, mean_w, mean_b, logvar_w, logvar_b)
    mean, logvar, _ = _run(in_maps, trace=False)
    return mean, logvar


# revision 11
# speedup vs baseline: 1.3924x; 1.3924x over previous
"""Trainium2 Bass kernel for nn_BasicEncoder (embedding-lookup encoder).

reference math:
    counts[b, v] = histogram of the 512 token ids in row b          [B, V]
    h      = relu(counts @ enc1_w.T + enc1_b)                       [B, 16]
    mean   = h @ mean_w.T + mean_b                                  [B, 16]
    logvar = h @ logvar_w.T + logvar_b                              [B, 16]

Key identity: counts @ enc1_w.T == sum_s enc1_w[:, x[b, s]] — a gather-and-
sum of embedding-table columns; the [B, V] histogram is never materialized.

Device strategy (data-parallel over 8 NeuronCores, 256 rows x 512 tokens
per core):

  - The gather runs on the SWDGE DMA path (InstDMAGatherAnt): the Q7 cores
    generate one 64B descriptor per token (table row = 16 f32), and the 16
    DMA engines execute them HBM -> SBUF.  This replaces the Q7 ap_gather
    inner loop (the baseline), which is limited to ~2 indices per
    ~102-cycle SBUF read command.  The table lives in HBM as [V, 64] f32
    (256B row stride — the descriptor base-address step granularity); only
    bytes 0:64 of each row are fetched, so the pad is never read.
  - The runtime's SWDGE descriptor ring caps one gather instruction at
    ~65..120 descriptors per DMA engine; chunks of NI indices (NI/16+1
    descs/ring) stay under it.  Chunks rotate across the 4 SWDGE queues.
  - dma_gather's fixed dst layout: stream position i -> partition i%128,
    slot i//128.  Rows are mapped to partitions (128-row waves), so slot j
    of partition p holds token j's embedding for row p.  A strided DVE
    reduce over slots yields h-partials [128 rows, 16].
  - Tail per wave: PE transpose ([128,16] -> psum [16,128]), scalar Relu
    with per-partition bias enc1_b, then mean/logvar as one matmul each
    with a ones-row-augmented stationary (bias folded into the matmul).
  - Output [128 rows, 32] per wave, assembled host-side.
"""

import numpy as np

B, S, V, H, O = 2048, 512, 32128, 16, 16
NCORES = 8
RPC = B // NCORES  # rows per core (256)
P = 128
NWAVES = RPC // P  # 128-row waves per core (2)
JPC = 4  # token slots per chunk
CPW = S // JPC  # gather chunks per wave (64)
NCHUNK = NWAVES * CPW  # gather chunks per core (128)
NI = JPC * P  # indices per chunk (1024)
NIW = NI // 16  # wrapped idx columns per chunk (64)
TSTRIDE = 64  # table row stride in f32 elements (256B)
NQ = 4  # SWDGE queues to rotate across

_CACHE = {}


def _dma_gather_raw(gp, out_ap, in_ap, idxs_ap, num_idxs, elem_size, elem_step,
                    queue_num=0):
    """nc.gpsimd.dma_gather without the elem_size_bytes %256 assert.

    The 256B-multiple restriction in the bass helper is only required for
    transpose mode (the SBUF xbar sprays 256B); the non-transpose ucode path
    (dma_gather.cpp gen_descs) supports arbitrary packet sizes.  Only the
    descriptor base-address stride (elem_step) must be a 256B multiple.
    """
    import concourse.mybir as mybir

    assert idxs_ap.dtype == mybir.dt.int16
    assert in_ap.dtype == out_ap.dtype
    stride_bytes = elem_step * mybir.dt.size(in_ap.dtype)
    stride_bytes_256, rem = divmod(stride_bytes, 256)
    assert rem == 0 and 0 < stride_bytes_256 < 256
    _in_ap = gp.lower_ap_dma(in_ap, for_custom_bir_dma=True)
    inst = gp.add_instruction(
        mybir.InstDMAGatherAnt(
            name=gp.bass.get_next_instruction_name(),
            ins=[
                *_in_ap,
                gp.lower_ap(idxs_ap),
                gp.lower_val_access(gp.to_reg(num_idxs)),
            ],
            outs=[gp.lower_ap(out_ap)],
            transpose=False,
            num_idxs=num_idxs,
            elem_size=elem_size,
            stride_bytes_256=stride_bytes_256,
            gen_mode=0,
            single_packet=True,
            queue_num=queue_num,
            sbuf_tokens_per_rank=0,
            sbuf_free_dim_per_rank=0,
            sbuf_free_dim_pad_per_rank=0,
            sbuf_byte_offset=0,
        )
    )
    return inst


def _build_nc(repeat=1):
    import contextlib

    import concourse.bacc as bacc
    import concourse.bass as bass
    import concourse.mybir as mybir
    import concourse.tile as tile

    f32 = mybir.dt.float32
    i16 = mybir.dt.int16
    nc = bacc.Bacc(
        None,
        target_bir_lowering=False,
        dynamic_dma_scratch_size=65536,
        num_swdge_queues=NQ,
    )

    tbl_d = nc.dram_tensor("tbl64", [V, TSTRIDE], f32, kind="ExternalInput")
    xi_d = nc.dram_tensor("xi16", [P, NCHUNK * NIW], i16, kind="ExternalInput")
    b1_d = nc.dram_tensor("b1col", [H, 1], f32, kind="ExternalInput")
    ident_d = nc.dram_tensor("ident", [P, P], f32, kind="ExternalInput")
    ones_d = nc.dram_tensor("onesrow", [1, P], f32, kind="ExternalInput")
    wm_d = nc.dram_tensor("wmaug", [H + 1, O], f32, kind="ExternalInput")
    wl_d = nc.dram_tensor("wlaug", [H + 1, O], f32, kind="ExternalInput")
    out_d = nc.dram_tensor("out", [P, NWAVES * 2 * O], f32, kind="ExternalOutput")

    with tile.TileContext(nc) as tc:
        with (
            tc.tile_pool(name="sb", bufs=1) as pool,
            tc.tile_pool(name="gth", bufs=4) as gpool,
            tc.tile_pool(name="pst", bufs=2, space=bass.MemorySpace.PSUM) as ptpool,
            tc.tile_pool(name="ps", bufs=1, space=bass.MemorySpace.PSUM) as pspool,
            tc.For_i(0, repeat, 1) if repeat > 1 else contextlib.nullcontext(),
        ):
            # -- small constants ------------------------------------------
            b1_sb = pool.tile([H, 1], f32)
            nc.sync.dma_start(b1_sb[:], b1_d[:])
            ident_sb = pool.tile([P, P], f32)
            nc.sync.dma_start(ident_sb[:], ident_d[:])
            wm_sb = pool.tile([H + 1, O], f32)
            nc.sync.dma_start(wm_sb[:], wm_d[:])
            wl_sb = pool.tile([H + 1, O], f32)
            nc.sync.dma_start(wl_sb[:], wl_d[:])

            # index streams, 8 DMAs so early gathers start promptly
            NIDMA = 8
            xi_sb = pool.tile([P, NCHUNK * NIW], i16)
            cols = NCHUNK * NIW // NIDMA
            for k in range(NIDMA):
                nc.sync.dma_start(
                    xi_sb[:, k * cols : (k + 1) * cols],
                    xi_d[:, k * cols : (k + 1) * cols],
                )

            # ones rows for the bias-augmented stationaries (one per wave)
            hrT = [pool.tile([H + 1, P], f32, name=f"hrT{w}") for w in range(NWAVES)]
            for w in range(NWAVES):
                nc.sync.dma_start(hrT[w][H : H + 1, :], ones_d[:])

            # -- gather + reduce ------------------------------------------
            hpart = [
                pool.tile([P, CPW * H], f32, name=f"hpart{w}") for w in range(NWAVES)
            ]
            for k in range(NCHUNK):
                w, c = divmod(k, CPW)
                gath = gpool.tile([P, JPC * H], f32)
                _dma_gather_raw(
                    nc.gpsimd,
                    out_ap=gath[:],
                    in_ap=tbl_d[:, 0:H],
                    idxs_ap=xi_sb[:, k * NIW : (k + 1) * NIW],
                    num_idxs=NI,
                    elem_size=H,
                    elem_step=TSTRIDE,
                    queue_num=k % NQ,
                )
                nc.vector.tensor_reduce(
                    out=hpart[w][:, c * H : (c + 1) * H],
                    in_=gath[:].rearrange("p (j s) -> p s j", s=H),
                    axis=mybir.AxisListType.X,
                    op=mybir.AluOpType.add,
                )

            # -- per-wave tail --------------------------------------------
            out_sb = pool.tile([P, NWAVES * 2 * O], f32)
            om_ps = pspool.tile([P, NWAVES * 2 * O], f32)
            for w in range(NWAVES):
                hsum = pool.tile([P, H], f32)
                nc.vector.tensor_reduce(
                    out=hsum[:],
                    in_=hpart[w][:].rearrange("p (c s) -> p s c", s=H),
                    axis=mybir.AxisListType.X,
                    op=mybir.AluOpType.add,
                )
                ht_ps = ptpool.tile([H, P], f32)
                nc.tensor.transpose(ht_ps[:], hsum[:], ident_sb[:])
                nc.scalar.activation(
                    out=hrT[w][0:H, :],
                    in_=ht_ps[:],
                    func=mybir.ActivationFunctionType.Relu,
                    bias=b1_sb[:],
                )
                nc.tensor.matmul(
                    om_ps[:, (2 * w) * O : (2 * w + 1) * O], hrT[w][:], wm_sb[:]
                )
                nc.tensor.matmul(
                    om_ps[:, (2 * w + 1) * O : (2 * w + 2) * O], hrT[w][:], wl_sb[:]
                )

            nc.vector.tensor_scalar(
                out=out_sb[:],
                in0=om_ps[:],
                scalar1=0.0,
                scalar2=None,
                op0=mybir.AluOpType.add,
            )
            nc.sync.dma_start(out_d[:], out_sb[:])

    nc.compile()
    return nc


def _get_nc(repeat=1):
    key = ("nc", repeat)
    if key not in _CACHE:
        _CACHE[key] = _build_nc(repeat)
    return _CACHE[key]


def _prep_inputs(x, enc1_w, enc1_b, mean_w, mean_b, logvar_w, logvar_b):
    x = np.asarray(x)
    assert x.shape == (B, S)
    # stream position i of chunk (w, c): slot j = i // 128, partition
    # p = i % 128 -> token x[core_base + w*128 + p, c*JPC + j]
    xs = x.astype(np.int16).reshape(NCORES, NWAVES, P, CPW, JPC)  # [n, w, p, c, j]
    stream = xs.transpose(0, 1, 3, 4, 2)  # [n, w, c, j, p]
    stream = stream.reshape(NCORES, NCHUNK, NI)
    # idx i of a chunk is read from [16g + i%16, i//16] (any 16-partition
    # group g; the stream is replicated across all 8 groups)
    wrapped = stream.reshape(NCORES, NCHUNK, NIW, 16).transpose(0, 3, 1, 2)
    xi16 = np.tile(
        wrapped.reshape(NCORES, 1, 16, NCHUNK * NIW), (1, 8, 1, 1)
    ).reshape(NCORES, P, NCHUNK * NIW)
    xi16 = np.ascontiguousarray(xi16)

    tbl64 = np.zeros((V, TSTRIDE), dtype=np.float32)
    tbl64[:, :H] = np.asarray(enc1_w, dtype=np.float32).T  # [V, H]

    b1col = np.asarray(enc1_b, dtype=np.float32)[:, None].copy()
    ident = np.eye(P, dtype=np.float32)
    onesrow = np.ones((1, P), dtype=np.float32)

    def aug(wt, bias):  # [O, H] -> [H+1, O] with bias row
        a = np.empty((H + 1, O), dtype=np.float32)
        a[:H] = np.asarray(wt, dtype=np.float32).T
        a[H] = np.asarray(bias, dtype=np.float32)
        return a

    wmaug = aug(mean_w, mean_b)
    wlaug = aug(logvar_w, logvar_b)
    return [
        {
            "tbl64": tbl64,
            "xi16": xi16[c],
            "b1col": b1col,
            "ident": ident,
            "onesrow": onesrow,
            "wmaug": wmaug,
            "wlaug": wlaug,
        }
        for c in range(NCORES)
    ]


def _unscramble(out_core):
    # out_core [128, NWAVES*2*O]: partition p, wave w -> row w*128 + p
    o = out_core.reshape(P, NWAVES, 2, O)  # [p, w, {m,l}, o]
    o = o.transpose(2, 1, 0, 3)  # [{m,l}, w, p, o]
    return o.reshape(2, NWAVES * P, O)  # rows r = w*128 + p


def _run(in_maps, trace=False, repeat=1):
    from concourse.bass_utils import run_bass_kernel_spmd

    nc = _get_nc(repeat)
    core_ids = list(range(NCORES))
    res = run_bass_kernel_spmd(nc, in_maps, core_ids, trace=trace)
    mean = np.empty((B, O), dtype=np.float32)
    logvar = np.empty((B, O), dtype=np.float32)
    for c in core_ids:
        ml = _unscramble(res.results[c]["out"])
        mean[c * RPC : (c + 1) * RPC] = ml[0]
        logvar[c * RPC : (c + 1) * RPC] = ml[1]
    return mean, logvar, res


def kernel(x, enc1_w, enc1_b, mean_w, mean_b, logvar_w, logvar_b):
    in_maps = _prep_inputs(x, enc1_w, enc1_b, mean_w, mean_b, logvar_w, logvar_b)
    mean, logvar, _ = _run(in_maps, trace=False)
    return mean, logvar


# revision 12
# speedup vs baseline: 153.9966x; 110.5977x over previous
"""Trainium2 Bass kernel for nn_BasicEncoder (embedding-lookup encoder).

reference math:
    counts[b, v] = histogram of the 512 token ids in row b          [B, V]
    h      = relu(counts @ enc1_w.T + enc1_b)                       [B, 16]
    mean   = h @ mean_w.T + mean_b                                  [B, 16]
    logvar = h @ logvar_w.T + logvar_b                              [B, 16]

Key identity: counts @ enc1_w.T  ==  sum_s enc1_w[:, x[b, s]], i.e. a
gather-and-sum of embedding-table columns.  The [B, V] histogram is never
materialized.

Device strategy (data-parallel over 8 NeuronCores, batch-sharded, 256
rows x 512 tokens per core):

  - The [16, V] table is replicated 8x down the SBUF partitions
    ([128, V] f32, 125.5KB/partition).  The GpSimd ap_gather instruction
    gathers per 16-partition group with that group's own index stream,
    so 8 batch rows are gathered concurrently (one per Q7 core):
        gath[16g + h, j*512 + s] = enc1_w[h, x[row(g, j), s]]
  - A vector-engine reduce over the free dim sums each row's 512
    embeddings: hall[16g + h, n] = h[row(g, n), h].
  - enc1_b rides the scalar-engine Relu as a per-partition bias
    (partition p has h-dim p%16).
  - mean/logvar = one matmul each against a block-diagonal (8 x [16,16])
    weight matrix, keeping the (group, h) partition layout; output biases
    added as per-partition scalars.  Host unscrambles [128, 32] -> [256, 32].
"""

import numpy as np

B, S, V, H, O = 2048, 512, 32128, 16, 16
NCORES = 8
RPC = B // NCORES  # rows per core (256)
P = 128
G = 8  # partition groups (= Q7 cores)
SLOTS = RPC // G  # rows per group (32)
RPG = 4  # rows gathered per group per ap_gather instruction (HW-tuned)
NINST = SLOTS // RPG  # ap_gather instructions per core (8)
NI = RPG * S  # indices per group per instruction (2048)
NIW = NI // 16  # wrapped idx columns per instruction (128)

_CACHE = {}


def _build_nc(repeat=1):
    import contextlib

    import concourse.bacc as bacc
    import concourse.bass as bass
    import concourse.mybir as mybir
    import concourse.tile as tile

    f32 = mybir.dt.float32
    i16 = mybir.dt.int16
    nc = bacc.Bacc(None, target_bir_lowering=False)

    tbl_d = nc.dram_tensor("tblr", [H, V], f32, kind="ExternalInput")
    xi_d = nc.dram_tensor("xi16", [P, NINST * NIW], i16, kind="ExternalInput")
    b1_d = nc.dram_tensor("b1rep", [P, 1], f32, kind="ExternalInput")
    wm_d = nc.dram_tensor("wmbd", [P, P], f32, kind="ExternalInput")
    wl_d = nc.dram_tensor("wlbd", [P, P], f32, kind="ExternalInput")
    bm_d = nc.dram_tensor("bmrep", [P, 1], f32, kind="ExternalInput")
    bl_d = nc.dram_tensor("blrep", [P, 1], f32, kind="ExternalInput")
    out_d = nc.dram_tensor("out", [P, 2 * SLOTS], f32, kind="ExternalOutput")

    with tile.TileContext(nc) as tc:
        with (
            tc.tile_pool(name="sb", bufs=1) as pool,
            tc.tile_pool(name="gth", bufs=3) as gpool,
            tc.tile_pool(name="ps", bufs=1, space=bass.MemorySpace.PSUM) as pspool,
            tc.For_i(0, repeat, 1) if repeat > 1 else contextlib.nullcontext(),
        ):
            tbl_sb = pool.tile([P, V], f32)
            for g in range(G):
                nc.sync.dma_start(tbl_sb[g * H : (g + 1) * H, :], tbl_d[:])
            xi_sb = pool.tile([P, NINST * NIW], i16)
            nc.sync.dma_start(xi_sb[:], xi_d[:])
            b1_sb = pool.tile([P, 1], f32)
            nc.sync.dma_start(b1_sb[:], b1_d[:])
            wm_sb = pool.tile([P, P], f32)
            nc.sync.dma_start(wm_sb[:], wm_d[:])
            wl_sb = pool.tile([P, P], f32)
            nc.sync.dma_start(wl_sb[:], wl_d[:])
            bm_sb = pool.tile([P, 1], f32)
            nc.sync.dma_start(bm_sb[:], bm_d[:])
            bl_sb = pool.tile([P, 1], f32)
            nc.sync.dma_start(bl_sb[:], bl_d[:])

            hall = pool.tile([P, SLOTS], f32)
            for k in range(NINST):
                gath = gpool.tile([P, NI], f32)
                nc.gpsimd.ap_gather(
                    out_ap=gath[:],
                    in_ap=tbl_sb[:],
                    idxs_ap=xi_sb[:, k * NIW : (k + 1) * NIW],
                    channels=P,
                    num_elems=V,
                    d=1,
                    num_idxs=NI,
                )
                nc.vector.tensor_reduce(
                    out=hall[:, k * RPG : (k + 1) * RPG],
                    in_=gath[:].rearrange("p (r s) -> p r s", s=S),
                    axis=mybir.AxisListType.X,
                    op=mybir.AluOpType.add,
                )

            hr = pool.tile([P, SLOTS], f32)
            nc.scalar.activation(
                out=hr[:],
                in_=hall[:],
                func=mybir.ActivationFunctionType.Relu,
                bias=b1_sb[:],
            )

            om_ps = pspool.tile([P, SLOTS], f32)
            nc.tensor.matmul(om_ps[:], wm_sb[:], hr[:])
            ol_ps = pspool.tile([P, SLOTS], f32)
            nc.tensor.matmul(ol_ps[:], wl_sb[:], hr[:])

            o_sb = pool.tile([P, 2 * SLOTS], f32)
            nc.vector.tensor_scalar(
                out=o_sb[:, :SLOTS],
                in0=om_ps[:],
                scalar1=bm_sb[:],
                scalar2=None,
                op0=mybir.AluOpType.add,
            )
            nc.vector.tensor_scalar(
                out=o_sb[:, SLOTS:],
                in0=ol_ps[:],
                scalar1=bl_sb[:],
                scalar2=None,
                op0=mybir.AluOpType.add,
            )
            nc.sync.dma_start(out_d[:], o_sb[:])

    nc.compile()
    return nc


def _get_nc(repeat=1):
    key = ("nc", repeat)
    if key not in _CACHE:
        _CACHE[key] = _build_nc(repeat)
    return _CACHE[key]


def _prep_inputs(x, enc1_w, enc1_b, mean_w, mean_b, logvar_w, logvar_b):
    x = np.asarray(x)
    assert x.shape == (B, S)
    # row r of core c = global row c*RPC + r; within a core, row r is
    # handled by group g = r % G at slot n = r // G; instruction k covers
    # slots k*RPG .. k*RPG+RPG-1.
    xs = x.astype(np.int16).reshape(NCORES, SLOTS, G, S)  # [c, n, g, s]
    # per (c, k, g): index stream = concat over j (slot n=k*RPG+j) of tokens
    stream = xs.transpose(0, 2, 1, 3).reshape(NCORES, G, NINST, NI)  # [c,g,k,i]
    # wrapped: idx i -> [16g + i%16, i//16]
    wrapped = stream.reshape(NCORES, G, NINST, NIW, 16).transpose(0, 1, 4, 2, 3)
    xi16 = np.ascontiguousarray(
        wrapped.reshape(NCORES, G * 16, NINST * NIW)
    )  # [c, 128, NINST*NIW]

    tblr = np.ascontiguousarray(np.asarray(enc1_w, dtype=np.float32))  # [H, V]
    pidx = np.arange(P) % H
    b1rep = np.asarray(enc1_b, dtype=np.float32)[pidx][:, None].copy()
    bmrep = np.asarray(mean_b, dtype=np.float32)[pidx][:, None].copy()
    blrep = np.asarray(logvar_b, dtype=np.float32)[pidx][:, None].copy()

    # block-diagonal stationary: w_bd[16g+h, 16g+o] = w[o, h]
    def blockdiag(w):
        w = np.asarray(w, dtype=np.float32)  # [O, H]
        bd = np.zeros((P, P), dtype=np.float32)
        for g in range(G):
            bd[g * H : (g + 1) * H, g * O : (g + 1) * O] = w.T
        return bd

    wmbd = blockdiag(mean_w)
    wlbd = blockdiag(logvar_w)
    return [
        {
            "tblr": tblr,
            "xi16": xi16[c],
            "b1rep": b1rep,
            "wmbd": wmbd,
            "wlbd": wlbd,
            "bmrep": bmrep,
            "blrep": blrep,
        }
        for c in range(NCORES)
    ]


def _unscramble(out_core):
    # out_core [128, 2*SLOTS]: partition 16g+o, col n -> row n*G+g
    o = out_core.reshape(G, O, 2, SLOTS)  # [g, o, {m,l}, n]
    o = o.transpose(2, 3, 0, 1)  # [{m,l}, n, g, o]
    return o.reshape(2, SLOTS * G, O)  # rows r = n*G + g


def _run(in_maps, trace=False, repeat=1):
    from concourse.bass_utils import run_bass_kernel_spmd

    nc = _get_nc(repeat)
    core_ids = list(range(NCORES))
    res = run_bass_kernel_spmd(nc, in_maps, core_ids, trace=trace)
    mean = np.empty((B, O), dtype=np.float32)
    logvar = np.empty((B, O), dtype=np.float32)
    for c in core_ids:
        ml = _unscramble(res.results[c]["out"])
        mean[c * RPC : (c + 1) * RPC] = ml[0]
        logvar[c * RPC : (c + 1) * RPC] = ml[1]
    return mean, logvar, res


def kernel(x, enc1_w, enc1_b, mean_w, mean_b, logvar_w, logvar_b):
    in_maps = _prep_inputs(x, enc1_w, enc1_b, mean_w, mean_b, logvar_w, logvar_b)
    mean, logvar, _ = _run(in_maps, trace=False)
    return mean, logvar

